# revision 25
# baseline (speedup 1.0000x reference)
# Triplet-margin loss kernel for Trainium2 (Bass/Tile), batch-sharded
# across 8 NeuronCores.
#
# reference math (torch F.pairwise_distance semantics):
#   d_ap[b,p] = || anc[b] - pos[b,p] + eps ||_2
#   d_an[b,n] = || anc[b] - neg[b,n] + eps ||_2
#   loss = mean_{b,p,n} max(d_ap[b,p] - d_an[b,n] + margin, 0)
# (eps=1e-6 shifts d^2 by ~1e-4 out of ~2048 -> ~5e-8 relative on d;
#  dropped here, far below the 2e-2 tolerance.)
#
# v7 design. The kernel is bound by the x-stream (24MB fp32 HBM reads
# per core, ~60-74us at the observed 340-400GB/s); engines and the
# head/tail are arranged to hide under it:
#  - x and anc are cast fp32->bf16 during the DMA itself (SWDGE /
#    nc.gpsimd.dma_start casts inline; HBM reads unchanged). anc goes
#    via HWDGE+DVE-cast so its 2MB overlaps the SWDGE spin-up.
#  - one batched TT per chunk computes u = x - a for ALL its slices
#    (bf16 2x mode + broadcast AP for a; ~0.57us/slice at width 4).
#  - per slice, ONE square-reduce of u: 16 on ACT (activation Square w/
#    accum, ~1.7us incl accumulator read), 8 on DVE (stt u*u w/ accum,
#    ~1.4us); both engines sit ~55us/core, under the stream.
#  - chunks are [4,4,4,4] neg + [4,2,1,1] pos; the 1-slice tail chunks
#    mean only ~3.7us of work remains after the last DMA byte.
#  - the (p,n) combine is 2 wide ops via broadcast APs:
#    diff[p,j,n] = d_ap[j] - d_an[n]  (TT subtract, [P,8,16])
#    osb[:,t]    = sum relu(diff + margin)  (stt add/max w/ sum-accum)
#  - a dummy Sqrt is issued first so ACT loads the sqrt_and_others
#    table set (which also holds Square) exactly once.
# Each core returns per-partition partial sums [128, NT]; the host sums
# and scales.

import numpy as np

import concourse.bacc as bacc
import concourse.mybir as mybir
import concourse.tile as tile
from concourse import bass_utils

B, Z = 2048, 1024
NUM_POS, NUM_NEG = 8, 16
NJ = NUM_POS + NUM_NEG
MARGIN = 1.0
N_CORES = 8
BL = B // N_CORES  # 256 rows of anc per core
P = 128
NT = BL // P  # 2 batch-tiles per core

F32 = mybir.dt.float32
BF16 = mybir.dt.bfloat16
AF = mybir.ActivationFunctionType
OP = mybir.AluOpType

# per-tile chunk schedule: (kind, slice offset within kind, width)
# neg streams first; pos last with 2-slice tail chunks for a short
# post-stream drain.
CHUNKS = [
    ("neg", 0, 4),
    ("neg", 4, 4),
    ("neg", 8, 4),
    ("neg", 12, 4),
    ("pos", 0, 4),
    ("pos", 4, 2),
    ("pos", 6, 2),
]

# square-reduce engine per (chunk, slice): 7 on DVE, 17 on ACT per
# tile; each tail chunk is 1 ACT + 1 DVE so the drain after the last
# DMA byte runs on both engines in parallel.
_DVE_SET = {
    (0, 3),
    (1, 3),
    (2, 3),
    (3, 3),
    (4, 3),
    (5, 1),
    (6, 1),
}


def _emit(tc, nc, anc, pos, neg, out):
    v = nc.vector
    act = nc.scalar
    gp = nc.gpsimd
    pos2 = pos.rearrange("(b j) z -> b (j z)", j=NUM_POS)  # [BL, 8*Z]
    neg2 = neg.rearrange("(b j) z -> b (j z)", j=NUM_NEG)  # [BL, 16*Z]
    with (
        tc.tile_pool(name="xp4", bufs=6) as xp4,
        tc.tile_pool(name="xp2", bufs=3) as xp2,
        tc.tile_pool(name="xp1", bufs=4) as xp1,
        tc.tile_pool(name="up1", bufs=8) as up1,
        tc.tile_pool(name="apool", bufs=1) as apool,
        tc.tile_pool(name="scp", bufs=1) as scp,
        tc.tile_pool(name="smp", bufs=2) as smp,
        tc.tile_pool(name="opool", bufs=1) as opool,
    ):
        osb = opool.tile([P, NT], F32, name="osb")
        dve_scr = scp.tile([P, Z], BF16, name="dve_scr")
        act_scr = scp.tile([P, Z], BF16, name="act_scr")
        relu_scr = scp.tile([P, NUM_POS * NUM_NEG], F32, name="relu_scr")
        zeros = scp.tile([P, NUM_POS * NUM_NEG], F32, name="zeros")
        v.memset(zeros[:, :], 0.0)
        dummy = scp.tile([P, 1], F32, name="dummy")

        # first ACT op: tiny Sqrt so the sqrt_and_others table set (which
        # also holds Square) loads exactly once, before data arrives.
        v.memset(dummy[:, :], 1.0)
        act.activation(dummy[:, :], dummy[:, :], AF.Sqrt)

        # anc for both tiles upfront: HWDGE fp32 load + DVE cast, so the
        # SWDGE queue starts with the first x chunk immediately (the 2MB
        # of anc reads also keep HBM busy during SWDGE spin-up).
        anc_bf = []
        anc_in = []
        for t in range(NT):
            a32 = apool.tile([P, Z], F32, name="anc_in")
            nc.sync.dma_start(a32[:, :], anc[t * P : (t + 1) * P, :])
            anc_in.append(a32)
        for t in range(NT):
            abf = apool.tile([P, Z], BF16, name="anc_bf")
            v.tensor_copy(abf[:, :], anc_in[t][:, :])
            anc_bf.append(abf)

        # issue all x chunk DMAs (cast fp32->bf16 in flight)
        chunks = {}
        for t in range(NT):
            b0 = t * P
            for ci, (kind, s0, w) in enumerate(CHUNKS):
                xpool = {4: xp4, 2: xp2, 1: xp1}[w]
                xt = xpool.tile([P, w * Z], BF16, name="xt")
                src2 = pos2 if kind == "pos" else neg2
                src = src2[b0 : b0 + P, s0 * Z : (s0 + w) * Z]
                gp.dma_start(xt[:, :], src)
                chunks[(t, ci)] = xt

        for t in range(NT):
            # d2 stays in SBUF: a PSUM accumulator tile makes the tile
            # scheduler serialize the cross-engine (ACT+DVE) column
            # writes, destroying all overlap.
            d2 = smp.tile([P, NJ], F32, name="d2")
            dt_ = smp.tile([P, NJ], F32, name="dt_")
            diff = smp.tile([P, NUM_POS * NUM_NEG], F32, name="diff")
            abf = anc_bf[t]

            for ci, (kind, s0, w) in enumerate(CHUNKS):
                xt = chunks[(t, ci)]
                for q in range(w):
                    jj = (s0 + q) if kind == "pos" else (NUM_POS + s0 + q)
                    xs = xt[:, q * Z : (q + 1) * Z]
                    # per-slice TT (not batched per chunk): the finer
                    # granularity hands u to ACT ~0.7us after each slice
                    # lands instead of ~2.3us after the chunk, which
                    # empirically beats the DVE cycles a batched TT saves.
                    ut = up1.tile([P, Z], BF16, name="ut")
                    v.tensor_tensor(
                        out=ut[:, :], in0=xs, in1=abf[:, :], op=OP.subtract
                    )
                    us = ut[:, :]
                    if (ci, q) in _DVE_SET:
                        v.scalar_tensor_tensor(
                            out=dve_scr[:, :],
                            in0=us,
                            scalar=1.0,
                            in1=us,
                            op0=OP.bypass,
                            op1=OP.mult,
                            accum_out=d2[:, jj : jj + 1],
                        )
                    else:
                        act.activation(
                            act_scr[:, :],
                            us,
                            AF.Square,
                            accum_out=d2[:, jj : jj + 1],
                        )

            act.activation(dt_[:, :], d2[:, :], AF.Sqrt)
            # diff[p, j, n] = d_ap[j] - d_an[n] via broadcast APs
            pos_v = (
                dt_[:, 0:NUM_POS]
                .rearrange("p (j o) -> p j o", o=1)
                .broadcast_to((P, NUM_POS, NUM_NEG))
            )
            neg_v = (
                dt_[:, NUM_POS:NJ]
                .rearrange("p (o n) -> p o n", o=1)
                .broadcast_to((P, NUM_POS, NUM_NEG))
            )
            diff_v = diff[:, :].rearrange("p (j n) -> p j n", j=NUM_POS)
            v.tensor_tensor(out=diff_v, in0=pos_v, in1=neg_v, op=OP.subtract)
            # osb[:,t] = sum_{j,n} relu(diff + margin); stt's accumulator
            # sums `out` regardless of op1 (tensor_scalar's follows op1).
            v.scalar_tensor_tensor(
                out=relu_scr[:, :],
                in0=diff[:, :],
                scalar=MARGIN,
                in1=zeros[:, :],
                op0=OP.add,
                op1=OP.max,
                accum_out=osb[:, t : t + 1],
            )
        nc.sync.dma_start(out[:, :], osb[:, :])


_NC_CACHE = None


def build():
    global _NC_CACHE
    if _NC_CACHE is None:
        nc = bacc.Bacc(
            "TRN2",
            target_bir_lowering=False,
            debug=False,
            num_devices=N_CORES,
            num_swdge_queues=4,
            dynamic_dma_scratch_size=32768,
        )
        anc = nc.dram_tensor("anc", (BL, Z), F32, kind="ExternalInput").ap()
        pos = nc.dram_tensor("pos", (BL * NUM_POS, Z), F32, kind="ExternalInput").ap()
        neg = nc.dram_tensor("neg", (BL * NUM_NEG, Z), F32, kind="ExternalInput").ap()
        out = nc.dram_tensor("out", (P, NT), F32, kind="ExternalOutput").ap()
        with tile.TileContext(nc) as tc:
            _emit(tc, nc, anc, pos, neg, out)
        nc.compile()
        _NC_CACHE = nc
    return _NC_CACHE


def make_in_maps(anc_embedding, pos_embedding, neg_embedding):
    anc_embedding = np.asarray(anc_embedding, dtype=np.float32)
    pos_embedding = np.asarray(pos_embedding, dtype=np.float32)
    neg_embedding = np.asarray(neg_embedding, dtype=np.float32)
    in_maps = []
    for c in range(N_CORES):
        in_maps.append(
            {
                "anc": np.ascontiguousarray(anc_embedding[c * BL : (c + 1) * BL]),
                "pos": np.ascontiguousarray(
                    pos_embedding[c * BL * NUM_POS : (c + 1) * BL * NUM_POS]
                ),
                "neg": np.ascontiguousarray(
                    neg_embedding[c * BL * NUM_NEG : (c + 1) * BL * NUM_NEG]
                ),
            }
        )
    return in_maps


def combine(outs):
    # outs: list of [P, NT] per-core partial sums of relu(d_ap-d_an+margin)
    total = sum(o.astype(np.float64).sum() for o in outs)
    return np.float32(total / (B * NUM_POS * NUM_NEG))


def kernel(anc_embedding, pos_embedding, neg_embedding):
    nc = build()
    in_maps = make_in_maps(anc_embedding, pos_embedding, neg_embedding)
    res = bass_utils.run_bass_kernel_spmd(nc, in_maps, core_ids=list(range(N_CORES)))
    return combine([r["out"] for r in res.results])
